# revision 51
# baseline (speedup 1.0000x reference)
"""BoundaryMaxPooling Trainium2 kernel.

Reference computation (B=16, C2=512, T=Tf=126):
  - segment windows [s0,s1) / [e0,e1) derived from segments[0] only (batch-0 row)
  - out[b, c, t]      = max_{j in [s0(t), s1(t))} feature[b, c, j]       (c < 256)
  - out[b, 256+c, t]  = max_{j in [e0(t), e1(t))} feature[b, 256+c, j]

Device algorithm (per core, 2 batches, data-parallel over batch):
  Sparse-table (log-level) range max with j on SBUF partitions:
    L_0[j, c'] = feature^T   (c' = half*512 + b*256 + c, 1024 columns, bf16)
    L_{k+1}[j] = max(L_k[j], L_k[j + 2^k])   for j in [0, 127 - 2^{k+1})
  Partition shifts 1/2/4/8/16 are produced by the TensorEngine with exact
  one-hot band matrices (fp8e4 stationary x bf16 moving, fp32 PSUM); the
  shift by 32 (level 5 -> 6) is a direct partition-offset read on the DVE
  (offsets 0/32/64/96 are legal for compute engines), no matmul needed.
  Window max for window length L, k = floor(log2 L):
    out[t] = max(L_k[a(t)], L_k[b(t)]),  a = lo, b = hi - 2^k
  Both lookups are exact one-hot gather matmuls (fp8e4 one-hots) accumulated
  over levels in PSUM; a zero one-hot column contributes exact 0.  The final
  max of the two PSUM accumulators is a single DVE op per half writing the
  bf16 output tile.  Host precomputes all index matrices from segments[0]
  (replicated across cores), pre-transposes features per core (bf16), and
  reassembles/transposes the output; empty end-windows (e0 == -1) are
  data-independent and set to float32 min on the host, matching the
  reference.  All values stay exactly bf16 end-to-end on device (max never
  creates new values), so the only rounding is the host's fp32->bf16 cast.
"""

import os
import sys

import numpy as np

if os.path.isdir("/opt/trn_rl_repo") and "/opt/trn_rl_repo" not in sys.path:
    sys.path.insert(0, "/opt/trn_rl_repo")

import concourse.bass as bass  # noqa: E402
from concourse import bacc, mybir, tile  # noqa: E402
from concourse.bass_utils import run_bass_kernel_spmd  # noqa: E402

B, C2, T = 16, 512, 126
C = C2 // 2  # 256
NCORES = 8
BPC = B // NCORES  # batches per core = 2
CPRIME = BPC * C2  # 1024 columns per core
NLEV = 7
KS = [127 - (1 << k) for k in range(NLEV)]  # valid rows of level k
NSHIFT = 6  # PE shift matmuls for levels 1..6 (compute engines cannot read
# SBUF at partition offsets other than 0 for tensor_tensor: both-SB inputs
# must share a base partition, so even the shift-by-32 needs the PE)
WCOL = 128  # every stationary matrix padded to 128 columns (enables FWL)

F32 = mybir.dt.float32
BF16 = mybir.dt.bfloat16
F8 = mybir.dt.float8e4
U8 = mybir.dt.uint8
MAX = mybir.AluOpType.max

_CACHE = {}

# test.py hooks: set TRACE=True before calling kernel() to capture a profile.
TRACE = False
LAST_RESULTS = None


FT_BYTES = CPRIME * 2  # 2048 B of bf16 feature row per partition


def _w8_layout():
    """Byte offsets of each fp8 matrix inside the packed uint8 tensors.

    DMA throughput here is packet-bound (~175ns per partition-row packet
    regardless of size), so inputs are packed into as few fat-row tensors
    as possible and bitcast on device:
      pk0 [T, FT_BYTES + 6*WCOL]: ft (bf16 bytes) + the 6 shift matrices —
          everything the level chain needs, lands first.
      pk1 [T, 16*WCOL]: gather one-hots, levels 0-3.
      pk2 [T, 12*WCOL]: gather one-hots, levels 4-6.
    All three go on the same HWDGE ring in order, so pk0 gets the full
    16-engine bandwidth and the gathers stream in during the chain.
    Returns ({key: (tensor_idx, byte_off)}, [nbytes0, nbytes1, nbytes2]).
    """
    offs = {}
    nbytes = [FT_BYTES, 0, 0]
    for k in range(NSHIFT):
        offs[("sh", k)] = (0, nbytes[0])
        nbytes[0] += WCOL
    for k in range(NLEV):
        ti = 1 if k < 4 else 2
        for gi in range(2):
            for h in range(2):
                offs[("g", gi, h, k)] = (ti, nbytes[ti])
                nbytes[ti] += WCOL
    return offs, nbytes


def _build_module():
    nc = bacc.Bacc(None, target_bir_lowering=False, debug=False)

    offs, nbytes = _w8_layout()
    pk_ins = [
        nc.dram_tensor(f"pk{i}", [T, nbytes[i]], U8, kind="ExternalInput")
        for i in range(3)
    ]
    out = nc.dram_tensor("out", [T, CPRIME], BF16, kind="ExternalOutput")

    with tile.TileContext(nc) as tc:
        with (
            tc.tile_pool(name="lv", bufs=1) as lvp,
            tc.tile_pool(name="gw", bufs=1) as gwp,
            tc.tile_pool(name="acc", bufs=1, space=bass.MemorySpace.PSUM) as accp,
            tc.tile_pool(name="shp", bufs=2, space=bass.MemorySpace.PSUM) as shpp,
            tc.tile_pool(name="shc", bufs=2) as shcp,
        ):
            pk = [
                gwp.tile([T, nbytes[i]], U8, name=f"pk{i}") for i in range(3)
            ]
            # All three on the sync ring, in order: pk0 (ft + shifts) gets
            # the full 16-SDMA-engine bandwidth first; the gather one-hots
            # stream in behind it while the chain runs.
            for i in range(3):
                nc.sync.dma_start(out=pk[i][:, :], in_=pk_ins[i][:, :])

            ft = pk[0][:, 0:FT_BYTES].bitcast(BF16)

            def sh_ap(k):
                ti, o = offs[("sh", k)]
                return pk[ti][0 : KS[k], o : o + WCOL].bitcast(F8)

            def g_ap(gi, h, k):
                ti, o = offs[("g", gi, h, k)]
                return pk[ti][0 : KS[k], o : o + WCOL].bitcast(F8)

            L = [None] + [
                lvp.tile([KS[k], CPRIME], BF16, name=f"L{k}")[:, :]
                for k in range(1, NLEV)
            ]

            def L_ap(k, h, rows):
                if k == 0:
                    return ft[0:rows, h * 512 : (h + 1) * 512]
                return L[k][0:rows, h * 512 : (h + 1) * 512]

            p_acc = [
                accp.tile([128, CPRIME], F32, name=f"pacc{gi}") for gi in range(2)
            ]

            # PE warmup: HAM throttles the PE to half clock until it has been
            # continuously busy ~3.4us. A few matmuls on a zeroed tile bridge
            # the input-DMA wait so real matmuls start immediately and reach
            # full clock early. (Too many would serialize ahead of real work.)
            # Inputs cannot land before ~11us (engine preamble gates the DMA
            # trigger, plus ~2us DMA path latency). The HAM un-throttles the
            # PE only after a ~3.4us CONTIGUOUS busy window, so bridge the
            # wait with a few fp32 N=512 warmup matmuls: fp32 runs LOW+HIGH
            # double passes (~1.7us per matmul cold), giving one long
            # gap-free busy stretch exactly like the original working
            # baseline. bf16 warmups proved fragile here (scheduler-inserted
            # stalls between them kept resetting the HAM window).
            wzs = gwp.tile([128, 128], F32, name="wzs")
            wzm = gwp.tile([128, 512], F32, name="wzm")
            nc.vector.memset(wzs[:, :], 0.0)
            nc.vector.memset(wzm[:, :], 0.0)
            for _ in range(3):
                nc.tensor.matmul(
                    p_acc[0][0:128, 0:512],
                    wzs[:, :],
                    wzm[:, :],
                    start=True,
                    stop=True,
                )

            # The shift chain is the critical path: emit each level's shift
            # matmuls first, then the previous level's gathers as PE filler
            # (they only need the already-built L_k, so they never gate the
            # chain). DVE max(h) runs while PE shifts the other half.
            def emit_gathers(k, gi, hs=(0, 1)):
                for h in hs:
                    sl = slice(h * 512, (h + 1) * 512)
                    nc.tensor.matmul(
                        p_acc[gi][:, sl],
                        g_ap(gi, h, k),
                        L_ap(k, h, KS[k]),
                        start=(k == 0),
                        stop=(k == NLEV - 1),
                    )

            # The chain (shift matmul -> DVE max per level) is the critical
            # path. Only the gi=0 gathers run inline (PSUM bank 0); the gi=1
            # gathers are deferred past the chain into their own bank,
            # halving the PE work that paces each level. (An ACT-staged
            # PSUM->SBUF copy before the DVE max was tried and is NOT
            # faster: the fp32 source blocks the scalar engine's 2x mode,
            # so the copy costs as much as the DVE op it would speed up.)
            if True:
                for k in range(NSHIFT):
                    shp = shpp.tile(
                        [128, CPRIME], F32, name=f"shp{k}", tag="shp"
                    )
                    for h in range(2):
                        sl = slice(h * 512, (h + 1) * 512)
                        nc.tensor.matmul(
                            shp[:, sl],
                            sh_ap(k),
                            L_ap(k, h, KS[k]),
                            start=True,
                            stop=True,
                        )
                        nc.vector.tensor_max(
                            L[k + 1][:, sl],
                            L_ap(k, h, KS[k + 1]),
                            shp[0 : KS[k + 1], sl],
                        )
                    emit_gathers(k, gi=0)
                emit_gathers(NSHIFT, gi=0)
            for k in range(NLEV):
                emit_gathers(k, gi=1, hs=(0,))
            for k in range(NLEV):
                emit_gathers(k, gi=1, hs=(1,))


            # TensorTensor may read only one PSUM operand: stage acc0 through
            # SBUF via the (otherwise idle) scalar engine, casting to bf16.
            # High priority so the ACT copy runs as soon as PSUM bank 0's
            # accumulation closes, overlapping the deferred gi=1 gathers.
            s1t = gwp.tile([T, CPRIME], BF16, name="s1t")
            ot = gwp.tile([T, CPRIME], BF16, name="ot")
            with tc.high_priority():
                for half in range(2):
                    sl = slice(half * 512, (half + 1) * 512)
                    nc.scalar.copy(out=s1t[:, sl], in_=p_acc[0][0:T, sl])
                    nc.vector.tensor_max(
                        ot[:, sl], s1t[:, sl], p_acc[1][0:T, sl]
                    )
                    eng = nc.sync if half == 0 else nc.scalar
                    eng.dma_start(out=out[:, sl], in_=ot[:, sl])

    nc.compile()
    return nc


def _host_windows(segments):
    """Replicates the reference's index math on segments[0]. Returns per half
    (lo, hi) clamped windows plus the empty mask."""
    seg = np.clip(segments.astype(np.float32), 0.0, 125.0)
    row = seg[0]  # [T, 4]
    s0 = np.floor(row[:, 0]).astype(np.int32)
    s1 = np.ceil(row[:, 1]).astype(np.int32)
    s1 = np.where(s0 == s1, s1 + 1, s1)
    e0 = np.floor(row[:, 2]).astype(np.int32)
    e1 = np.ceil(row[:, 3]).astype(np.int32)
    e0 = np.where(e0 == e1, e0 - 1, e0)

    halves = []
    for lo, hi in ((s0, s1), (e0, e1)):
        lo_c = np.maximum(lo, 0)
        hi_c = np.minimum(hi, T)
        empty = lo_c >= hi_c
        halves.append((lo_c, hi_c, empty))
    return halves


def _host_pk(segments):
    """Packed uint8 input tensors (fp8 one-hot bytes; pk0 also carries ft
    bytes which the caller fills per core)."""
    halves = _host_windows(segments)
    offs, nbytes = _w8_layout()
    one = mybir.dt.np(F8)(1.0).view(np.uint8)  # fp8e4 bit pattern of 1.0
    pk = [np.zeros((T, n), np.uint8) for n in nbytes]
    for k in range(NSHIFT):
        ti, o = offs[("sh", k)]
        s = 1 << k
        for j in range(KS[k + 1]):
            pk[ti][j + s, o + j] = one
    for h, (lo, hi, empty) in enumerate(halves):
        for t in range(T):
            if empty[t]:
                continue
            ln = int(hi[t] - lo[t])
            k = ln.bit_length() - 1
            a = int(lo[t])
            b = int(hi[t]) - (1 << k)
            ta, oa = offs[("g", 0, h, k)]
            tb, ob = offs[("g", 1, h, k)]
            pk[ta][a, oa + t] = one
            pk[tb][b, ob + t] = one
    return pk, halves


def _shard_feature(feature):
    """Core i gets batches [2i, 2i+2) as bf16 [T, CPRIME] with
    c' = half*512 + local_batch*256 + channel_within_half."""
    bf = mybir.dt.np(BF16)
    fts = []
    for i in range(NCORES):
        pair = feature[BPC * i : BPC * (i + 1)]
        arr = pair.reshape(BPC, 2, C, T)  # [b, h, c, j]
        arr = np.ascontiguousarray(arr.transpose(3, 1, 0, 2).reshape(T, CPRIME))
        fts.append(arr.astype(bf))
    return fts


def _unshard(results, halves):
    out = np.empty((B, C2, T), np.float32)
    for i in range(NCORES):
        r = np.asarray(results[i]["out"]).astype(np.float32)  # [T, CPRIME]
        arr = r.reshape(T, 2, BPC, C).transpose(2, 1, 3, 0)  # [b, h, c, t]
        out[BPC * i : BPC * (i + 1)] = arr.reshape(BPC, C2, T)
    neg = np.finfo(np.float32).min
    for h, (_, _, empty) in enumerate(halves):
        if empty.any():
            out[:, h * C : (h + 1) * C, empty] = neg
    return out


def kernel(feature, segments):
    global LAST_RESULTS
    feature = np.ascontiguousarray(feature, dtype=np.float32)
    segments = np.ascontiguousarray(segments, dtype=np.float32)

    if "nc" not in _CACHE:
        _CACHE["nc"] = _build_module()
    nc = _CACHE["nc"]

    pk, halves = _host_pk(segments)
    fts = _shard_feature(feature)

    in_maps = []
    for i in range(NCORES):
        pk0 = pk[0].copy()
        pk0[:, 0:FT_BYTES] = fts[i].view(np.uint8)
        in_maps.append({"pk0": pk0, "pk1": pk[1], "pk2": pk[2]})

    res = run_bass_kernel_spmd(nc, in_maps, list(range(NCORES)), trace=TRACE)
    LAST_RESULTS = res
    return _unshard(res.results, halves)


# revision 52
# speedup vs baseline: 1.0202x; 1.0202x over previous
"""BoundaryMaxPooling Trainium2 kernel.

Reference computation (B=16, C2=512, T=Tf=126):
  - segment windows [s0,s1) / [e0,e1) derived from segments[0] only (batch-0 row)
  - out[b, c, t]      = max_{j in [s0(t), s1(t))} feature[b, c, j]       (c < 256)
  - out[b, 256+c, t]  = max_{j in [e0(t), e1(t))} feature[b, 256+c, j]

Device algorithm (per core, 2 batches, data-parallel over batch):
  Sparse-table (log-level) range max with j on SBUF partitions:
    L_0[j, c'] = feature^T   (c' = half*512 + b*256 + c, 1024 columns, bf16)
    L_{k+1}[j] = max(L_k[j], L_k[j + 2^k])   for j in [0, 127 - 2^{k+1})
  Partition shifts 1/2/4/8/16 are produced by the TensorEngine with exact
  one-hot band matrices (fp8e4 stationary x bf16 moving, fp32 PSUM); the
  shift by 32 (level 5 -> 6) is a direct partition-offset read on the DVE
  (offsets 0/32/64/96 are legal for compute engines), no matmul needed.
  Window max for window length L, k = floor(log2 L):
    out[t] = max(L_k[a(t)], L_k[b(t)]),  a = lo, b = hi - 2^k
  Both lookups are exact one-hot gather matmuls (fp8e4 one-hots) accumulated
  over levels in PSUM; a zero one-hot column contributes exact 0.  The final
  max of the two PSUM accumulators is a single DVE op per half writing the
  bf16 output tile.  Host precomputes all index matrices from segments[0]
  (replicated across cores), pre-transposes features per core (bf16), and
  reassembles/transposes the output; empty end-windows (e0 == -1) are
  data-independent and set to float32 min on the host, matching the
  reference.  All values stay exactly bf16 end-to-end on device (max never
  creates new values), so the only rounding is the host's fp32->bf16 cast.
"""

import os
import sys

import numpy as np

if os.path.isdir("/opt/trn_rl_repo") and "/opt/trn_rl_repo" not in sys.path:
    sys.path.insert(0, "/opt/trn_rl_repo")

import concourse.bass as bass  # noqa: E402
from concourse import bacc, mybir, tile  # noqa: E402
from concourse.bass_utils import run_bass_kernel_spmd  # noqa: E402

B, C2, T = 16, 512, 126
C = C2 // 2  # 256
NCORES = 8
BPC = B // NCORES  # batches per core = 2
CPRIME = BPC * C2  # 1024 columns per core
NLEV = 7
KS = [127 - (1 << k) for k in range(NLEV)]  # valid rows of level k
NSHIFT = 6  # PE shift matmuls for levels 1..6 (compute engines cannot read
# SBUF at partition offsets other than 0 for tensor_tensor: both-SB inputs
# must share a base partition, so even the shift-by-32 needs the PE)
WCOL = 128  # every stationary matrix padded to 128 columns (enables FWL)

F32 = mybir.dt.float32
BF16 = mybir.dt.bfloat16
F8 = mybir.dt.float8e4
U8 = mybir.dt.uint8
MAX = mybir.AluOpType.max

_CACHE = {}

# test.py hooks: set TRACE=True before calling kernel() to capture a profile.
TRACE = False
LAST_RESULTS = None


FT_BYTES = CPRIME * 2  # 2048 B of bf16 feature row per partition


def _w8_layout():
    """Byte offsets of each fp8 matrix inside the packed uint8 tensors.

    DMA throughput here is packet-bound (~175ns per partition-row packet
    regardless of size), so inputs are packed into as few fat-row tensors
    as possible and bitcast on device:
      pk0 [T, FT_BYTES + 6*WCOL]: ft (bf16 bytes) + the 6 shift matrices —
          everything the level chain needs, lands first.
      pk1 [T, 16*WCOL]: gather one-hots, levels 0-3.
      pk2 [T, 12*WCOL]: gather one-hots, levels 4-6.
    All three go on the same HWDGE ring in order, so pk0 gets the full
    16-engine bandwidth and the gathers stream in during the chain.
    Returns ({key: (tensor_idx, byte_off)}, [nbytes0, nbytes1, nbytes2]).
    """
    offs = {}
    nbytes = [FT_BYTES, 0, 0]
    for k in range(NSHIFT):
        offs[("sh", k)] = (0, nbytes[0])
        nbytes[0] += WCOL
    for k in range(NLEV):
        ti = 1 if k < 4 else 2
        for gi in range(2):
            for h in range(2):
                offs[("g", gi, h, k)] = (ti, nbytes[ti])
                nbytes[ti] += WCOL
    return offs, nbytes


def _build_module():
    nc = bacc.Bacc(None, target_bir_lowering=False, debug=False)

    offs, nbytes = _w8_layout()
    pk_ins = [
        nc.dram_tensor(f"pk{i}", [T, nbytes[i]], U8, kind="ExternalInput")
        for i in range(3)
    ]
    out = nc.dram_tensor("out", [T, CPRIME], BF16, kind="ExternalOutput")

    with tile.TileContext(nc) as tc:
        with (
            tc.tile_pool(name="lv", bufs=1) as lvp,
            tc.tile_pool(name="gw", bufs=1) as gwp,
            tc.tile_pool(name="acc", bufs=1, space=bass.MemorySpace.PSUM) as accp,
            tc.tile_pool(name="shp", bufs=2, space=bass.MemorySpace.PSUM) as shpp,
            tc.tile_pool(name="shc", bufs=2) as shcp,
        ):
            pk = [
                gwp.tile([T, nbytes[i]], U8, name=f"pk{i}") for i in range(3)
            ]
            # All three on the sync ring, in order: pk0 (ft + shifts) gets
            # the full 16-SDMA-engine bandwidth first; the gather one-hots
            # stream in behind it while the chain runs.
            for i in range(3):
                nc.sync.dma_start(out=pk[i][:, :], in_=pk_ins[i][:, :])

            ft = pk[0][:, 0:FT_BYTES].bitcast(BF16)

            def sh_ap(k):
                ti, o = offs[("sh", k)]
                return pk[ti][0 : KS[k], o : o + WCOL].bitcast(F8)

            def g_ap(gi, h, k):
                ti, o = offs[("g", gi, h, k)]
                return pk[ti][0 : KS[k], o : o + WCOL].bitcast(F8)

            L = [None] + [
                lvp.tile([KS[k], CPRIME], BF16, name=f"L{k}")[:, :]
                for k in range(1, NLEV)
            ]

            def L_ap(k, h, rows):
                if k == 0:
                    return ft[0:rows, h * 512 : (h + 1) * 512]
                return L[k][0:rows, h * 512 : (h + 1) * 512]

            p_acc = [
                accp.tile([128, CPRIME], F32, name=f"pacc{gi}") for gi in range(2)
            ]

            # PE warmup: HAM throttles the PE to half clock until it has been
            # continuously busy ~3.4us. A few matmuls on a zeroed tile bridge
            # the input-DMA wait so real matmuls start immediately and reach
            # full clock early. (Too many would serialize ahead of real work.)
            # Inputs cannot land before ~11us (engine preamble gates the DMA
            # trigger, plus ~2us DMA path latency). The HAM un-throttles the
            # PE only after a ~3.4us CONTIGUOUS busy window, so bridge the
            # wait with a few fp32 N=512 warmup matmuls: fp32 runs LOW+HIGH
            # double passes (~1.7us per matmul cold), giving one long
            # gap-free busy stretch exactly like the original working
            # baseline. bf16 warmups proved fragile here (scheduler-inserted
            # stalls between them kept resetting the HAM window).
            wzs = gwp.tile([128, 128], F32, name="wzs")
            wzm = gwp.tile([128, 512], F32, name="wzm")
            nc.vector.memset(wzs[:, :], 0.0)
            nc.vector.memset(wzm[:, :], 0.0)
            for _ in range(4):
                nc.tensor.matmul(
                    p_acc[0][0:128, 0:512],
                    wzs[:, :],
                    wzm[:, :],
                    start=True,
                    stop=True,
                )

            # The shift chain is the critical path: emit each level's shift
            # matmuls first, then the previous level's gathers as PE filler
            # (they only need the already-built L_k, so they never gate the
            # chain). DVE max(h) runs while PE shifts the other half.
            def emit_gathers(k, gi, hs=(0, 1)):
                for h in hs:
                    sl = slice(h * 512, (h + 1) * 512)
                    nc.tensor.matmul(
                        p_acc[gi][:, sl],
                        g_ap(gi, h, k),
                        L_ap(k, h, KS[k]),
                        start=(k == 0),
                        stop=(k == NLEV - 1),
                    )

            # The chain (shift matmul -> DVE max per level) is the critical
            # path. Only the gi=0 gathers run inline (PSUM bank 0); the gi=1
            # gathers are deferred past the chain into their own bank,
            # halving the PE work that paces each level. (An ACT-staged
            # PSUM->SBUF copy before the DVE max was tried and is NOT
            # faster: the fp32 source blocks the scalar engine's 2x mode,
            # so the copy costs as much as the DVE op it would speed up.)
            if True:
                for k in range(NSHIFT):
                    shp = shpp.tile(
                        [128, CPRIME], F32, name=f"shp{k}", tag="shp"
                    )
                    for h in range(2):
                        sl = slice(h * 512, (h + 1) * 512)
                        nc.tensor.matmul(
                            shp[:, sl],
                            sh_ap(k),
                            L_ap(k, h, KS[k]),
                            start=True,
                            stop=True,
                        )
                        nc.vector.tensor_max(
                            L[k + 1][:, sl],
                            L_ap(k, h, KS[k + 1]),
                            shp[0 : KS[k + 1], sl],
                        )
                    emit_gathers(k, gi=0)
                emit_gathers(NSHIFT, gi=0)
            for k in range(NLEV):
                emit_gathers(k, gi=1)


            # TensorTensor may read only one PSUM operand: stage acc0 through
            # SBUF via the (otherwise idle) scalar engine, casting to bf16.
            # High priority so the ACT copy runs as soon as PSUM bank 0's
            # accumulation closes, overlapping the deferred gi=1 gathers.
            s1t = gwp.tile([T, CPRIME], BF16, name="s1t")
            ot = gwp.tile([T, CPRIME], BF16, name="ot")
            with tc.high_priority():
                for half in range(2):
                    sl = slice(half * 512, (half + 1) * 512)
                    nc.scalar.copy(out=s1t[:, sl], in_=p_acc[0][0:T, sl])
                    nc.vector.tensor_max(
                        ot[:, sl], s1t[:, sl], p_acc[1][0:T, sl]
                    )
                    eng = nc.sync if half == 0 else nc.scalar
                    eng.dma_start(out=out[:, sl], in_=ot[:, sl])

    nc.compile()
    return nc


def _host_windows(segments):
    """Replicates the reference's index math on segments[0]. Returns per half
    (lo, hi) clamped windows plus the empty mask."""
    seg = np.clip(segments.astype(np.float32), 0.0, 125.0)
    row = seg[0]  # [T, 4]
    s0 = np.floor(row[:, 0]).astype(np.int32)
    s1 = np.ceil(row[:, 1]).astype(np.int32)
    s1 = np.where(s0 == s1, s1 + 1, s1)
    e0 = np.floor(row[:, 2]).astype(np.int32)
    e1 = np.ceil(row[:, 3]).astype(np.int32)
    e0 = np.where(e0 == e1, e0 - 1, e0)

    halves = []
    for lo, hi in ((s0, s1), (e0, e1)):
        lo_c = np.maximum(lo, 0)
        hi_c = np.minimum(hi, T)
        empty = lo_c >= hi_c
        halves.append((lo_c, hi_c, empty))
    return halves


def _host_pk(segments):
    """Packed uint8 input tensors (fp8 one-hot bytes; pk0 also carries ft
    bytes which the caller fills per core)."""
    halves = _host_windows(segments)
    offs, nbytes = _w8_layout()
    one = mybir.dt.np(F8)(1.0).view(np.uint8)  # fp8e4 bit pattern of 1.0
    pk = [np.zeros((T, n), np.uint8) for n in nbytes]
    for k in range(NSHIFT):
        ti, o = offs[("sh", k)]
        s = 1 << k
        for j in range(KS[k + 1]):
            pk[ti][j + s, o + j] = one
    for h, (lo, hi, empty) in enumerate(halves):
        for t in range(T):
            if empty[t]:
                continue
            ln = int(hi[t] - lo[t])
            k = ln.bit_length() - 1
            a = int(lo[t])
            b = int(hi[t]) - (1 << k)
            ta, oa = offs[("g", 0, h, k)]
            tb, ob = offs[("g", 1, h, k)]
            pk[ta][a, oa + t] = one
            pk[tb][b, ob + t] = one
    return pk, halves


def _shard_feature(feature):
    """Core i gets batches [2i, 2i+2) as bf16 [T, CPRIME] with
    c' = half*512 + local_batch*256 + channel_within_half."""
    bf = mybir.dt.np(BF16)
    fts = []
    for i in range(NCORES):
        pair = feature[BPC * i : BPC * (i + 1)]
        arr = pair.reshape(BPC, 2, C, T)  # [b, h, c, j]
        arr = np.ascontiguousarray(arr.transpose(3, 1, 0, 2).reshape(T, CPRIME))
        fts.append(arr.astype(bf))
    return fts


def _unshard(results, halves):
    out = np.empty((B, C2, T), np.float32)
    for i in range(NCORES):
        r = np.asarray(results[i]["out"]).astype(np.float32)  # [T, CPRIME]
        arr = r.reshape(T, 2, BPC, C).transpose(2, 1, 3, 0)  # [b, h, c, t]
        out[BPC * i : BPC * (i + 1)] = arr.reshape(BPC, C2, T)
    neg = np.finfo(np.float32).min
    for h, (_, _, empty) in enumerate(halves):
        if empty.any():
            out[:, h * C : (h + 1) * C, empty] = neg
    return out


def kernel(feature, segments):
    global LAST_RESULTS
    feature = np.ascontiguousarray(feature, dtype=np.float32)
    segments = np.ascontiguousarray(segments, dtype=np.float32)

    if "nc" not in _CACHE:
        _CACHE["nc"] = _build_module()
    nc = _CACHE["nc"]

    pk, halves = _host_pk(segments)
    fts = _shard_feature(feature)

    in_maps = []
    for i in range(NCORES):
        pk0 = pk[0].copy()
        pk0[:, 0:FT_BYTES] = fts[i].view(np.uint8)
        in_maps.append({"pk0": pk0, "pk1": pk[1], "pk2": pk[2]})

    res = run_bass_kernel_spmd(nc, in_maps, list(range(NCORES)), trace=TRACE)
    LAST_RESULTS = res
    return _unshard(res.results, halves)


# revision 53
# speedup vs baseline: 1.0332x; 1.0127x over previous
"""BoundaryMaxPooling Trainium2 kernel.

Reference computation (B=16, C2=512, T=Tf=126):
  - segment windows [s0,s1) / [e0,e1) derived from segments[0] only (batch-0 row)
  - out[b, c, t]      = max_{j in [s0(t), s1(t))} feature[b, c, j]       (c < 256)
  - out[b, 256+c, t]  = max_{j in [e0(t), e1(t))} feature[b, 256+c, j]

Device algorithm (per core, 2 batches, data-parallel over batch):
  Sparse-table (log-level) range max with j on SBUF partitions:
    L_0[j, c'] = feature^T   (c' = half*512 + b*256 + c, 1024 columns, bf16)
    L_{k+1}[j] = max(L_k[j], L_k[j + 2^k])   for j in [0, 127 - 2^{k+1})
  Partition shifts 1/2/4/8/16 are produced by the TensorEngine with exact
  one-hot band matrices (fp8e4 stationary x bf16 moving, fp32 PSUM); the
  shift by 32 (level 5 -> 6) is a direct partition-offset read on the DVE
  (offsets 0/32/64/96 are legal for compute engines), no matmul needed.
  Window max for window length L, k = floor(log2 L):
    out[t] = max(L_k[a(t)], L_k[b(t)]),  a = lo, b = hi - 2^k
  Both lookups are exact one-hot gather matmuls (fp8e4 one-hots) accumulated
  over levels in PSUM; a zero one-hot column contributes exact 0.  The final
  max of the two PSUM accumulators is a single DVE op per half writing the
  bf16 output tile.  Host precomputes all index matrices from segments[0]
  (replicated across cores), pre-transposes features per core (bf16), and
  reassembles/transposes the output; empty end-windows (e0 == -1) are
  data-independent and set to float32 min on the host, matching the
  reference.  All values stay exactly bf16 end-to-end on device (max never
  creates new values), so the only rounding is the host's fp32->bf16 cast.
"""

import os
import sys

import numpy as np

if os.path.isdir("/opt/trn_rl_repo") and "/opt/trn_rl_repo" not in sys.path:
    sys.path.insert(0, "/opt/trn_rl_repo")

import concourse.bass as bass  # noqa: E402
from concourse import bacc, mybir, tile  # noqa: E402
from concourse.bass_utils import run_bass_kernel_spmd  # noqa: E402

B, C2, T = 16, 512, 126
C = C2 // 2  # 256
NCORES = 8
BPC = B // NCORES  # batches per core = 2
CPRIME = BPC * C2  # 1024 columns per core
NLEV = 7
KS = [127 - (1 << k) for k in range(NLEV)]  # valid rows of level k
NSHIFT = 6  # PE shift matmuls for levels 1..6 (compute engines cannot read
# SBUF at partition offsets other than 0 for tensor_tensor: both-SB inputs
# must share a base partition, so even the shift-by-32 needs the PE)
WCOL = 128  # every stationary matrix padded to 128 columns (enables FWL)

F32 = mybir.dt.float32
BF16 = mybir.dt.bfloat16
F8 = mybir.dt.float8e4
U8 = mybir.dt.uint8
MAX = mybir.AluOpType.max

_CACHE = {}

# test.py hooks: set TRACE=True before calling kernel() to capture a profile.
TRACE = False
LAST_RESULTS = None


FT_BYTES = CPRIME * 2  # 2048 B of bf16 feature row per partition


def _w8_layout():
    """Byte offsets of each fp8 matrix inside the packed uint8 tensors.

    DMA throughput here is packet-bound (~175ns per partition-row packet
    regardless of size), so inputs are packed into as few fat-row tensors
    as possible and bitcast on device:
      pk0 [T, FT_BYTES + 6*WCOL]: ft (bf16 bytes) + the 6 shift matrices —
          everything the level chain needs, lands first.
      pk1 [T, 16*WCOL]: gather one-hots, levels 0-3.
      pk2 [T, 12*WCOL]: gather one-hots, levels 4-6.
    All three go on the same HWDGE ring in order, so pk0 gets the full
    16-engine bandwidth and the gathers stream in during the chain.
    Returns ({key: (tensor_idx, byte_off)}, [nbytes0, nbytes1, nbytes2]).
    """
    offs = {}
    nbytes = [FT_BYTES, 0, 0]
    for k in range(NSHIFT):
        offs[("sh", k)] = (0, nbytes[0])
        nbytes[0] += WCOL
    for k in range(NLEV):
        ti = 1 if k < 4 else 2
        for gi in range(2):
            for h in range(2):
                offs[("g", gi, h, k)] = (ti, nbytes[ti])
                nbytes[ti] += WCOL
    return offs, nbytes


def _build_module():
    nc = bacc.Bacc(None, target_bir_lowering=False, debug=False)

    offs, nbytes = _w8_layout()
    pk_ins = [
        nc.dram_tensor(f"pk{i}", [T, nbytes[i]], U8, kind="ExternalInput")
        for i in range(3)
    ]
    out = nc.dram_tensor("out", [T, CPRIME], BF16, kind="ExternalOutput")

    with tile.TileContext(nc) as tc:
        with (
            tc.tile_pool(name="lv", bufs=1) as lvp,
            tc.tile_pool(name="gw", bufs=1) as gwp,
            tc.tile_pool(name="acc", bufs=1, space=bass.MemorySpace.PSUM) as accp,
            tc.tile_pool(name="shp", bufs=2, space=bass.MemorySpace.PSUM) as shpp,
            tc.tile_pool(name="shc", bufs=2) as shcp,
        ):
            pk = [
                gwp.tile([T, nbytes[i]], U8, name=f"pk{i}") for i in range(3)
            ]
            # All three on the sync ring, in order: pk0 (ft + shifts) gets
            # the full 16-SDMA-engine bandwidth first; the gather one-hots
            # stream in behind it while the chain runs.
            for i in range(3):
                nc.sync.dma_start(out=pk[i][:, :], in_=pk_ins[i][:, :])

            ft = pk[0][:, 0:FT_BYTES].bitcast(BF16)

            def sh_ap(k):
                ti, o = offs[("sh", k)]
                return pk[ti][0 : KS[k], o : o + WCOL].bitcast(F8)

            def g_ap(gi, h, k):
                ti, o = offs[("g", gi, h, k)]
                return pk[ti][0 : KS[k], o : o + WCOL].bitcast(F8)

            L = [None] + [
                lvp.tile([KS[k], CPRIME], BF16, name=f"L{k}")[:, :]
                for k in range(1, NLEV)
            ]

            def L_ap(k, h, rows):
                if k == 0:
                    return ft[0:rows, h * 512 : (h + 1) * 512]
                return L[k][0:rows, h * 512 : (h + 1) * 512]

            p_acc = [
                accp.tile([128, CPRIME], F32, name=f"pacc{gi}") for gi in range(2)
            ]

            # PE warmup: HAM throttles the PE to half clock until it has been
            # continuously busy ~3.4us. A few matmuls on a zeroed tile bridge
            # the input-DMA wait so real matmuls start immediately and reach
            # full clock early. (Too many would serialize ahead of real work.)
            # Inputs cannot land before ~11us (engine preamble gates the DMA
            # trigger, plus ~2us DMA path latency). The HAM un-throttles the
            # PE only after a ~3.4us CONTIGUOUS busy window, so bridge the
            # wait with a few fp32 N=512 warmup matmuls: fp32 runs LOW+HIGH
            # double passes (~1.7us per matmul cold), giving one long
            # gap-free busy stretch exactly like the original working
            # baseline. bf16 warmups proved fragile here (scheduler-inserted
            # stalls between them kept resetting the HAM window).
            wzs = gwp.tile([128, 128], F32, name="wzs")
            wzm = gwp.tile([128, 512], F32, name="wzm")
            nc.vector.memset(wzs[:, :], 0.0)
            nc.vector.memset(wzm[:, :], 0.0)
            for _ in range(3):
                nc.tensor.matmul(
                    p_acc[0][0:128, 0:512],
                    wzs[:, :],
                    wzm[:, :],
                    start=True,
                    stop=True,
                )

            # The shift chain is the critical path: emit each level's shift
            # matmuls first, then the previous level's gathers as PE filler
            # (they only need the already-built L_k, so they never gate the
            # chain). DVE max(h) runs while PE shifts the other half.
            def emit_gathers(k, gi, hs=(0, 1)):
                for h in hs:
                    sl = slice(h * 512, (h + 1) * 512)
                    nc.tensor.matmul(
                        p_acc[gi][:, sl],
                        g_ap(gi, h, k),
                        L_ap(k, h, KS[k]),
                        start=(k == 0),
                        stop=(k == NLEV - 1),
                    )

            # The chain (shift matmul -> DVE max per level) is the critical
            # path. Only the gi=0 gathers run inline (PSUM bank 0); the gi=1
            # gathers are deferred past the chain into their own bank,
            # halving the PE work that paces each level. (An ACT-staged
            # PSUM->SBUF copy before the DVE max was tried and is NOT
            # faster: the fp32 source blocks the scalar engine's 2x mode,
            # so the copy costs as much as the DVE op it would speed up.)
            if True:
                for k in range(NSHIFT):
                    shp = shpp.tile(
                        [128, CPRIME], F32, name=f"shp{k}", tag="shp"
                    )
                    for h in range(2):
                        sl = slice(h * 512, (h + 1) * 512)
                        nc.tensor.matmul(
                            shp[:, sl],
                            sh_ap(k),
                            L_ap(k, h, KS[k]),
                            start=True,
                            stop=True,
                        )
                        nc.vector.tensor_max(
                            L[k + 1][:, sl],
                            L_ap(k, h, KS[k + 1]),
                            shp[0 : KS[k + 1], sl],
                        )
                    emit_gathers(k, gi=0)
                emit_gathers(NSHIFT, gi=0)
            for k in range(NLEV):
                emit_gathers(k, gi=1)


            # TensorTensor may read only one PSUM operand: stage acc0 through
            # SBUF via the (otherwise idle) scalar engine, casting to bf16.
            # High priority so the ACT copy runs as soon as PSUM bank 0's
            # accumulation closes, overlapping the deferred gi=1 gathers.
            s1t = gwp.tile([T, CPRIME], BF16, name="s1t")
            ot = gwp.tile([T, CPRIME], BF16, name="ot")
            with tc.high_priority():
                for half in range(2):
                    sl = slice(half * 512, (half + 1) * 512)
                    nc.scalar.copy(out=s1t[:, sl], in_=p_acc[0][0:T, sl])
                    nc.vector.tensor_max(
                        ot[:, sl], s1t[:, sl], p_acc[1][0:T, sl]
                    )
                    eng = nc.sync if half == 0 else nc.scalar
                    eng.dma_start(out=out[:, sl], in_=ot[:, sl])

    nc.compile()
    return nc


def _host_windows(segments):
    """Replicates the reference's index math on segments[0]. Returns per half
    (lo, hi) clamped windows plus the empty mask."""
    seg = np.clip(segments.astype(np.float32), 0.0, 125.0)
    row = seg[0]  # [T, 4]
    s0 = np.floor(row[:, 0]).astype(np.int32)
    s1 = np.ceil(row[:, 1]).astype(np.int32)
    s1 = np.where(s0 == s1, s1 + 1, s1)
    e0 = np.floor(row[:, 2]).astype(np.int32)
    e1 = np.ceil(row[:, 3]).astype(np.int32)
    e0 = np.where(e0 == e1, e0 - 1, e0)

    halves = []
    for lo, hi in ((s0, s1), (e0, e1)):
        lo_c = np.maximum(lo, 0)
        hi_c = np.minimum(hi, T)
        empty = lo_c >= hi_c
        halves.append((lo_c, hi_c, empty))
    return halves


def _host_pk(segments):
    """Packed uint8 input tensors (fp8 one-hot bytes; pk0 also carries ft
    bytes which the caller fills per core)."""
    halves = _host_windows(segments)
    offs, nbytes = _w8_layout()
    one = mybir.dt.np(F8)(1.0).view(np.uint8)  # fp8e4 bit pattern of 1.0
    pk = [np.zeros((T, n), np.uint8) for n in nbytes]
    for k in range(NSHIFT):
        ti, o = offs[("sh", k)]
        s = 1 << k
        for j in range(KS[k + 1]):
            pk[ti][j + s, o + j] = one
    for h, (lo, hi, empty) in enumerate(halves):
        for t in range(T):
            if empty[t]:
                continue
            ln = int(hi[t] - lo[t])
            k = ln.bit_length() - 1
            a = int(lo[t])
            b = int(hi[t]) - (1 << k)
            ta, oa = offs[("g", 0, h, k)]
            tb, ob = offs[("g", 1, h, k)]
            pk[ta][a, oa + t] = one
            pk[tb][b, ob + t] = one
    return pk, halves


def _shard_feature(feature):
    """Core i gets batches [2i, 2i+2) as bf16 [T, CPRIME] with
    c' = half*512 + local_batch*256 + channel_within_half."""
    bf = mybir.dt.np(BF16)
    fts = []
    for i in range(NCORES):
        pair = feature[BPC * i : BPC * (i + 1)]
        arr = pair.reshape(BPC, 2, C, T)  # [b, h, c, j]
        arr = np.ascontiguousarray(arr.transpose(3, 1, 0, 2).reshape(T, CPRIME))
        fts.append(arr.astype(bf))
    return fts


def _unshard(results, halves):
    out = np.empty((B, C2, T), np.float32)
    for i in range(NCORES):
        r = np.asarray(results[i]["out"]).astype(np.float32)  # [T, CPRIME]
        arr = r.reshape(T, 2, BPC, C).transpose(2, 1, 3, 0)  # [b, h, c, t]
        out[BPC * i : BPC * (i + 1)] = arr.reshape(BPC, C2, T)
    neg = np.finfo(np.float32).min
    for h, (_, _, empty) in enumerate(halves):
        if empty.any():
            out[:, h * C : (h + 1) * C, empty] = neg
    return out


def kernel(feature, segments):
    global LAST_RESULTS
    feature = np.ascontiguousarray(feature, dtype=np.float32)
    segments = np.ascontiguousarray(segments, dtype=np.float32)

    if "nc" not in _CACHE:
        _CACHE["nc"] = _build_module()
    nc = _CACHE["nc"]

    pk, halves = _host_pk(segments)
    fts = _shard_feature(feature)

    in_maps = []
    for i in range(NCORES):
        pk0 = pk[0].copy()
        pk0[:, 0:FT_BYTES] = fts[i].view(np.uint8)
        in_maps.append({"pk0": pk0, "pk1": pk[1], "pk2": pk[2]})

    res = run_bass_kernel_spmd(nc, in_maps, list(range(NCORES)), trace=TRACE)
    LAST_RESULTS = res
    return _unshard(res.results, halves)


# revision 54
# speedup vs baseline: 1.0473x; 1.0137x over previous
"""BoundaryMaxPooling Trainium2 kernel.

Reference computation (B=16, C2=512, T=Tf=126):
  - segment windows [s0,s1) / [e0,e1) derived from segments[0] only (batch-0 row)
  - out[b, c, t]      = max_{j in [s0(t), s1(t))} feature[b, c, j]       (c < 256)
  - out[b, 256+c, t]  = max_{j in [e0(t), e1(t))} feature[b, 256+c, j]

Device algorithm (per core, 2 batches, data-parallel over batch):
  Sparse-table (log-level) range max with j on SBUF partitions:
    L_0[j, c'] = feature^T   (c' = half*512 + b*256 + c, 1024 columns, bf16)
    L_{k+1}[j] = max(L_k[j], L_k[j + 2^k])   for j in [0, 127 - 2^{k+1})
  Partition shifts 1/2/4/8/16 are produced by the TensorEngine with exact
  one-hot band matrices (fp8e4 stationary x bf16 moving, fp32 PSUM); the
  shift by 32 (level 5 -> 6) is a direct partition-offset read on the DVE
  (offsets 0/32/64/96 are legal for compute engines), no matmul needed.
  Window max for window length L, k = floor(log2 L):
    out[t] = max(L_k[a(t)], L_k[b(t)]),  a = lo, b = hi - 2^k
  Both lookups are exact one-hot gather matmuls (fp8e4 one-hots) accumulated
  over levels in PSUM; a zero one-hot column contributes exact 0.  The final
  max of the two PSUM accumulators is a single DVE op per half writing the
  bf16 output tile.  Host precomputes all index matrices from segments[0]
  (replicated across cores), pre-transposes features per core (bf16), and
  reassembles/transposes the output; empty end-windows (e0 == -1) are
  data-independent and set to float32 min on the host, matching the
  reference.  All values stay exactly bf16 end-to-end on device (max never
  creates new values), so the only rounding is the host's fp32->bf16 cast.
"""

import os
import sys

import numpy as np

if os.path.isdir("/opt/trn_rl_repo") and "/opt/trn_rl_repo" not in sys.path:
    sys.path.insert(0, "/opt/trn_rl_repo")

import concourse.bass as bass  # noqa: E402
from concourse import bacc, mybir, tile  # noqa: E402
from concourse.bass_utils import run_bass_kernel_spmd  # noqa: E402

B, C2, T = 16, 512, 126
C = C2 // 2  # 256
NCORES = 8
BPC = B // NCORES  # batches per core = 2
CPRIME = BPC * C2  # 1024 columns per core
NLEV = 7
KS = [127 - (1 << k) for k in range(NLEV)]  # valid rows of level k
NSHIFT = 6  # PE shift matmuls for levels 1..6 (compute engines cannot read
# SBUF at partition offsets other than 0 for tensor_tensor: both-SB inputs
# must share a base partition, so even the shift-by-32 needs the PE)
WCOL = 128  # every stationary matrix padded to 128 columns (enables FWL)

F32 = mybir.dt.float32
BF16 = mybir.dt.bfloat16
F8 = mybir.dt.float8e4
U8 = mybir.dt.uint8
MAX = mybir.AluOpType.max

_CACHE = {}

# test.py hooks: set TRACE=True before calling kernel() to capture a profile.
TRACE = False
LAST_RESULTS = None


FT_BYTES = CPRIME * 2  # 2048 B of bf16 feature row per partition


def _w8_layout():
    """Byte offsets of each fp8 matrix inside the packed uint8 tensors.

    DMA throughput here is packet-bound (~175ns per partition-row packet
    regardless of size), so inputs are packed into as few fat-row tensors
    as possible and bitcast on device:
      pk0 [T, FT_BYTES + 6*WCOL]: ft (bf16 bytes) + the 6 shift matrices —
          everything the level chain needs, lands first.
      pk1 [T, 16*WCOL]: gather one-hots, levels 0-3.
      pk2 [T, 12*WCOL]: gather one-hots, levels 4-6.
    All three go on the same HWDGE ring in order, so pk0 gets the full
    16-engine bandwidth and the gathers stream in during the chain.
    Returns ({key: (tensor_idx, byte_off)}, [nbytes0, nbytes1, nbytes2]).
    """
    offs = {}
    nbytes = [FT_BYTES, 0, 0]
    for k in range(NSHIFT):
        offs[("sh", k)] = (0, nbytes[0])
        nbytes[0] += WCOL
    for k in range(NLEV):
        ti = 1 if k < 4 else 2
        for gi in range(2):
            for h in range(2):
                offs[("g", gi, h, k)] = (ti, nbytes[ti])
                nbytes[ti] += WCOL
    return offs, nbytes


def _build_module():
    nc = bacc.Bacc(None, target_bir_lowering=False, debug=False)

    offs, nbytes = _w8_layout()
    pk_ins = [
        nc.dram_tensor(f"pk{i}", [T, nbytes[i]], U8, kind="ExternalInput")
        for i in range(3)
    ]
    out = nc.dram_tensor("out", [T, CPRIME], BF16, kind="ExternalOutput")

    with tile.TileContext(nc) as tc:
        with (
            tc.tile_pool(name="lv", bufs=1) as lvp,
            tc.tile_pool(name="gw", bufs=1) as gwp,
            tc.tile_pool(name="acc", bufs=1, space=bass.MemorySpace.PSUM) as accp,
            tc.tile_pool(name="shp", bufs=2, space=bass.MemorySpace.PSUM) as shpp,
            tc.tile_pool(name="shc", bufs=2) as shcp,
        ):
            pk = [
                gwp.tile([T, nbytes[i]], U8, name=f"pk{i}") for i in range(3)
            ]
            # All three on the sync ring, in order: pk0 (ft + shifts) gets
            # the full 16-SDMA-engine bandwidth first; the gather one-hots
            # stream in behind it while the chain runs.
            for i in range(3):
                nc.sync.dma_start(out=pk[i][:, :], in_=pk_ins[i][:, :])

            ft = pk[0][:, 0:FT_BYTES].bitcast(BF16)

            def sh_ap(k):
                ti, o = offs[("sh", k)]
                return pk[ti][0 : KS[k], o : o + WCOL].bitcast(F8)

            def g_ap(gi, h, k):
                ti, o = offs[("g", gi, h, k)]
                return pk[ti][0 : KS[k], o : o + WCOL].bitcast(F8)

            L = [None] + [
                lvp.tile([KS[k], CPRIME], BF16, name=f"L{k}")[:, :]
                for k in range(1, NLEV)
            ]

            def L_ap(k, h, rows):
                if k == 0:
                    return ft[0:rows, h * 512 : (h + 1) * 512]
                return L[k][0:rows, h * 512 : (h + 1) * 512]

            p_acc = [
                accp.tile([128, CPRIME], F32, name=f"pacc{gi}") for gi in range(2)
            ]

            # PE warmup: HAM throttles the PE to half clock until it has been
            # continuously busy ~3.4us. A few matmuls on a zeroed tile bridge
            # the input-DMA wait so real matmuls start immediately and reach
            # full clock early. (Too many would serialize ahead of real work.)
            # Inputs cannot land before ~11us (engine preamble gates the DMA
            # trigger, plus ~2us DMA path latency). The HAM un-throttles the
            # PE only after a ~3.4us CONTIGUOUS busy window, so bridge the
            # wait with a few fp32 N=512 warmup matmuls: fp32 runs LOW+HIGH
            # double passes (~1.7us per matmul cold), giving one long
            # gap-free busy stretch exactly like the original working
            # baseline. bf16 warmups proved fragile here (scheduler-inserted
            # stalls between them kept resetting the HAM window).
            wzs = gwp.tile([128, 128], F32, name="wzs")
            wzm = gwp.tile([128, 512], F32, name="wzm")
            nc.vector.memset(wzs[:, :], 0.0)
            nc.vector.memset(wzm[:, :], 0.0)
            for _ in range(2):
                nc.tensor.matmul(
                    p_acc[0][0:128, 0:512],
                    wzs[:, :],
                    wzm[:, :],
                    start=True,
                    stop=True,
                )

            # The shift chain is the critical path: emit each level's shift
            # matmuls first, then the previous level's gathers as PE filler
            # (they only need the already-built L_k, so they never gate the
            # chain). DVE max(h) runs while PE shifts the other half.
            def emit_gathers(k, gi, hs=(0, 1)):
                for h in hs:
                    sl = slice(h * 512, (h + 1) * 512)
                    nc.tensor.matmul(
                        p_acc[gi][:, sl],
                        g_ap(gi, h, k),
                        L_ap(k, h, KS[k]),
                        start=(k == 0),
                        stop=(k == NLEV - 1),
                    )

            # The chain (shift matmul -> DVE max per level) is the critical
            # path. Only the gi=0 gathers run inline (PSUM bank 0); the gi=1
            # gathers are deferred past the chain into their own bank,
            # halving the PE work that paces each level. (An ACT-staged
            # PSUM->SBUF copy before the DVE max was tried and is NOT
            # faster: the fp32 source blocks the scalar engine's 2x mode,
            # so the copy costs as much as the DVE op it would speed up.)
            if True:
                for k in range(NSHIFT):
                    shp = shpp.tile(
                        [128, CPRIME], F32, name=f"shp{k}", tag="shp"
                    )
                    for h in range(2):
                        sl = slice(h * 512, (h + 1) * 512)
                        nc.tensor.matmul(
                            shp[:, sl],
                            sh_ap(k),
                            L_ap(k, h, KS[k]),
                            start=True,
                            stop=True,
                        )
                        nc.vector.tensor_max(
                            L[k + 1][:, sl],
                            L_ap(k, h, KS[k + 1]),
                            shp[0 : KS[k + 1], sl],
                        )
                    emit_gathers(k, gi=0)
                emit_gathers(NSHIFT, gi=0)
            for k in range(NLEV):
                emit_gathers(k, gi=1)


            # TensorTensor may read only one PSUM operand: stage acc0 through
            # SBUF via the (otherwise idle) scalar engine, casting to bf16.
            # High priority so the ACT copy runs as soon as PSUM bank 0's
            # accumulation closes, overlapping the deferred gi=1 gathers.
            s1t = gwp.tile([T, CPRIME], BF16, name="s1t")
            ot = gwp.tile([T, CPRIME], BF16, name="ot")
            with tc.high_priority():
                for half in range(2):
                    sl = slice(half * 512, (half + 1) * 512)
                    nc.scalar.copy(out=s1t[:, sl], in_=p_acc[0][0:T, sl])
                    nc.vector.tensor_max(
                        ot[:, sl], s1t[:, sl], p_acc[1][0:T, sl]
                    )
                    eng = nc.sync if half == 0 else nc.scalar
                    eng.dma_start(out=out[:, sl], in_=ot[:, sl])

    nc.compile()
    return nc


def _host_windows(segments):
    """Replicates the reference's index math on segments[0]. Returns per half
    (lo, hi) clamped windows plus the empty mask."""
    seg = np.clip(segments.astype(np.float32), 0.0, 125.0)
    row = seg[0]  # [T, 4]
    s0 = np.floor(row[:, 0]).astype(np.int32)
    s1 = np.ceil(row[:, 1]).astype(np.int32)
    s1 = np.where(s0 == s1, s1 + 1, s1)
    e0 = np.floor(row[:, 2]).astype(np.int32)
    e1 = np.ceil(row[:, 3]).astype(np.int32)
    e0 = np.where(e0 == e1, e0 - 1, e0)

    halves = []
    for lo, hi in ((s0, s1), (e0, e1)):
        lo_c = np.maximum(lo, 0)
        hi_c = np.minimum(hi, T)
        empty = lo_c >= hi_c
        halves.append((lo_c, hi_c, empty))
    return halves


def _host_pk(segments):
    """Packed uint8 input tensors (fp8 one-hot bytes; pk0 also carries ft
    bytes which the caller fills per core)."""
    halves = _host_windows(segments)
    offs, nbytes = _w8_layout()
    one = mybir.dt.np(F8)(1.0).view(np.uint8)  # fp8e4 bit pattern of 1.0
    pk = [np.zeros((T, n), np.uint8) for n in nbytes]
    for k in range(NSHIFT):
        ti, o = offs[("sh", k)]
        s = 1 << k
        for j in range(KS[k + 1]):
            pk[ti][j + s, o + j] = one
    for h, (lo, hi, empty) in enumerate(halves):
        for t in range(T):
            if empty[t]:
                continue
            ln = int(hi[t] - lo[t])
            k = ln.bit_length() - 1
            a = int(lo[t])
            b = int(hi[t]) - (1 << k)
            ta, oa = offs[("g", 0, h, k)]
            tb, ob = offs[("g", 1, h, k)]
            pk[ta][a, oa + t] = one
            pk[tb][b, ob + t] = one
    return pk, halves


def _shard_feature(feature):
    """Core i gets batches [2i, 2i+2) as bf16 [T, CPRIME] with
    c' = half*512 + local_batch*256 + channel_within_half."""
    bf = mybir.dt.np(BF16)
    fts = []
    for i in range(NCORES):
        pair = feature[BPC * i : BPC * (i + 1)]
        arr = pair.reshape(BPC, 2, C, T)  # [b, h, c, j]
        arr = np.ascontiguousarray(arr.transpose(3, 1, 0, 2).reshape(T, CPRIME))
        fts.append(arr.astype(bf))
    return fts


def _unshard(results, halves):
    out = np.empty((B, C2, T), np.float32)
    for i in range(NCORES):
        r = np.asarray(results[i]["out"]).astype(np.float32)  # [T, CPRIME]
        arr = r.reshape(T, 2, BPC, C).transpose(2, 1, 3, 0)  # [b, h, c, t]
        out[BPC * i : BPC * (i + 1)] = arr.reshape(BPC, C2, T)
    neg = np.finfo(np.float32).min
    for h, (_, _, empty) in enumerate(halves):
        if empty.any():
            out[:, h * C : (h + 1) * C, empty] = neg
    return out


def kernel(feature, segments):
    global LAST_RESULTS
    feature = np.ascontiguousarray(feature, dtype=np.float32)
    segments = np.ascontiguousarray(segments, dtype=np.float32)

    if "nc" not in _CACHE:
        _CACHE["nc"] = _build_module()
    nc = _CACHE["nc"]

    pk, halves = _host_pk(segments)
    fts = _shard_feature(feature)

    in_maps = []
    for i in range(NCORES):
        pk0 = pk[0].copy()
        pk0[:, 0:FT_BYTES] = fts[i].view(np.uint8)
        in_maps.append({"pk0": pk0, "pk1": pk[1], "pk2": pk[2]})

    res = run_bass_kernel_spmd(nc, in_maps, list(range(NCORES)), trace=TRACE)
    LAST_RESULTS = res
    return _unshard(res.results, halves)


# revision 55
# speedup vs baseline: 1.0859x; 1.0368x over previous
"""BoundaryMaxPooling Trainium2 kernel.

Reference computation (B=16, C2=512, T=Tf=126):
  - segment windows [s0,s1) / [e0,e1) derived from segments[0] only (batch-0 row)
  - out[b, c, t]      = max_{j in [s0(t), s1(t))} feature[b, c, j]       (c < 256)
  - out[b, 256+c, t]  = max_{j in [e0(t), e1(t))} feature[b, 256+c, j]

Device algorithm (per core, 2 batches, data-parallel over batch):
  Sparse-table (log-level) range max with j on SBUF partitions:
    L_0[j, c'] = feature^T   (c' = half*512 + b*256 + c, 1024 columns, bf16)
    L_{k+1}[j] = max(L_k[j], L_k[j + 2^k])   for j in [0, 127 - 2^{k+1})
  Partition shifts 1/2/4/8/16 are produced by the TensorEngine with exact
  one-hot band matrices (fp8e4 stationary x bf16 moving, fp32 PSUM); the
  shift by 32 (level 5 -> 6) is a direct partition-offset read on the DVE
  (offsets 0/32/64/96 are legal for compute engines), no matmul needed.
  Window max for window length L, k = floor(log2 L):
    out[t] = max(L_k[a(t)], L_k[b(t)]),  a = lo, b = hi - 2^k
  Both lookups are exact one-hot gather matmuls (fp8e4 one-hots) accumulated
  over levels in PSUM; a zero one-hot column contributes exact 0.  The final
  max of the two PSUM accumulators is a single DVE op per half writing the
  bf16 output tile.  Host precomputes all index matrices from segments[0]
  (replicated across cores), pre-transposes features per core (bf16), and
  reassembles/transposes the output; empty end-windows (e0 == -1) are
  data-independent and set to float32 min on the host, matching the
  reference.  All values stay exactly bf16 end-to-end on device (max never
  creates new values), so the only rounding is the host's fp32->bf16 cast.
"""

import os
import sys

import numpy as np

if os.path.isdir("/opt/trn_rl_repo") and "/opt/trn_rl_repo" not in sys.path:
    sys.path.insert(0, "/opt/trn_rl_repo")

import concourse.bass as bass  # noqa: E402
from concourse import bacc, mybir, tile  # noqa: E402
from concourse.bass_utils import run_bass_kernel_spmd  # noqa: E402

B, C2, T = 16, 512, 126
C = C2 // 2  # 256
NCORES = 8
BPC = B // NCORES  # batches per core = 2
CPRIME = BPC * C2  # 1024 columns per core
NLEV = 7
KS = [127 - (1 << k) for k in range(NLEV)]  # valid rows of level k
NSHIFT = 6  # PE shift matmuls for levels 1..6 (compute engines cannot read
# SBUF at partition offsets other than 0 for tensor_tensor: both-SB inputs
# must share a base partition, so even the shift-by-32 needs the PE)
WCOL = 128  # every stationary matrix padded to 128 columns (enables FWL)

F32 = mybir.dt.float32
BF16 = mybir.dt.bfloat16
F8 = mybir.dt.float8e4
U8 = mybir.dt.uint8
MAX = mybir.AluOpType.max

_CACHE = {}

# test.py hooks: set TRACE=True before calling kernel() to capture a profile.
TRACE = False
LAST_RESULTS = None


FT_BYTES = CPRIME * 2  # 2048 B of bf16 feature row per partition


def _w8_layout():
    """Byte offsets of each fp8 matrix inside the packed uint8 tensors.

    DMA throughput here is packet-bound (~175ns per partition-row packet
    regardless of size), so inputs are packed into as few fat-row tensors
    as possible and bitcast on device:
      pk0 [T, FT_BYTES + 6*WCOL]: ft (bf16 bytes) + the 6 shift matrices —
          everything the level chain needs, lands first.
      pk1 [T, 16*WCOL]: gather one-hots, levels 0-3.
      pk2 [T, 12*WCOL]: gather one-hots, levels 4-6.
    All three go on the same HWDGE ring in order, so pk0 gets the full
    16-engine bandwidth and the gathers stream in during the chain.
    Returns ({key: (tensor_idx, byte_off)}, [nbytes0, nbytes1, nbytes2]).
    """
    offs = {}
    nbytes = [FT_BYTES, 0, 0]
    for k in range(NSHIFT):
        offs[("sh", k)] = (0, nbytes[0])
        nbytes[0] += WCOL
    for k in range(NLEV):
        ti = 1 if k < 4 else 2
        for gi in range(2):
            for h in range(2):
                offs[("g", gi, h, k)] = (ti, nbytes[ti])
                nbytes[ti] += WCOL
    return offs, nbytes


def _build_module():
    nc = bacc.Bacc(None, target_bir_lowering=False, debug=False)

    offs, nbytes = _w8_layout()
    pk_ins = [
        nc.dram_tensor(f"pk{i}", [T, nbytes[i]], U8, kind="ExternalInput")
        for i in range(3)
    ]
    out = nc.dram_tensor("out", [T, CPRIME], BF16, kind="ExternalOutput")

    with tile.TileContext(nc) as tc:
        with (
            tc.tile_pool(name="lv", bufs=1) as lvp,
            tc.tile_pool(name="gw", bufs=1) as gwp,
            tc.tile_pool(name="acc", bufs=1, space=bass.MemorySpace.PSUM) as accp,
            tc.tile_pool(name="shp", bufs=2, space=bass.MemorySpace.PSUM) as shpp,
            tc.tile_pool(name="shc", bufs=2) as shcp,
        ):
            pk = [
                gwp.tile([T, nbytes[i]], U8, name=f"pk{i}") for i in range(3)
            ]
            # All three on the sync ring, in order: pk0 (ft + shifts) gets
            # the full 16-SDMA-engine bandwidth first; the gather one-hots
            # stream in behind it while the chain runs.
            for i in range(3):
                nc.sync.dma_start(out=pk[i][:, :], in_=pk_ins[i][:, :])

            ft = pk[0][:, 0:FT_BYTES].bitcast(BF16)

            def sh_ap(k):
                ti, o = offs[("sh", k)]
                return pk[ti][0 : KS[k], o : o + WCOL].bitcast(F8)

            def g_ap(gi, h, k):
                ti, o = offs[("g", gi, h, k)]
                return pk[ti][0 : KS[k], o : o + WCOL].bitcast(F8)

            L = [None] + [
                lvp.tile([KS[k], CPRIME], BF16, name=f"L{k}")[:, :]
                for k in range(1, NLEV)
            ]

            def L_ap(k, h, rows):
                if k == 0:
                    return ft[0:rows, h * 512 : (h + 1) * 512]
                return L[k][0:rows, h * 512 : (h + 1) * 512]

            p_acc = [
                accp.tile([128, CPRIME], F32, name=f"pacc{gi}") for gi in range(2)
            ]

            # PE warmup: HAM throttles the PE to half clock until it has been
            # continuously busy ~3.4us. A few matmuls on a zeroed tile bridge
            # the input-DMA wait so real matmuls start immediately and reach
            # full clock early. (Too many would serialize ahead of real work.)
            # Inputs cannot land before ~11us (engine preamble gates the DMA
            # trigger, plus ~2us DMA path latency). The HAM un-throttles the
            # PE only after a ~3.4us CONTIGUOUS busy window, so bridge the
            # wait with a few fp32 N=512 warmup matmuls: fp32 runs LOW+HIGH
            # double passes (~1.7us per matmul cold), giving one long
            # gap-free busy stretch exactly like the original working
            # baseline. bf16 warmups proved fragile here (scheduler-inserted
            # stalls between them kept resetting the HAM window).
            wzs = gwp.tile([128, 128], F32, name="wzs")
            wzm = gwp.tile([128, 512], F32, name="wzm")
            nc.vector.memset(wzs[:, :], 0.0)
            nc.vector.memset(wzm[:, :], 0.0)
            for _ in range(5):
                nc.tensor.matmul(
                    p_acc[0][0:128, 0:256],
                    wzs[:, :],
                    wzm[:, 0:256],
                    start=True,
                    stop=True,
                )

            # The shift chain is the critical path: emit each level's shift
            # matmuls first, then the previous level's gathers as PE filler
            # (they only need the already-built L_k, so they never gate the
            # chain). DVE max(h) runs while PE shifts the other half.
            def emit_gathers(k, gi, hs=(0, 1)):
                for h in hs:
                    sl = slice(h * 512, (h + 1) * 512)
                    nc.tensor.matmul(
                        p_acc[gi][:, sl],
                        g_ap(gi, h, k),
                        L_ap(k, h, KS[k]),
                        start=(k == 0),
                        stop=(k == NLEV - 1),
                    )

            # The chain (shift matmul -> DVE max per level) is the critical
            # path. Only the gi=0 gathers run inline (PSUM bank 0); the gi=1
            # gathers are deferred past the chain into their own bank,
            # halving the PE work that paces each level. (An ACT-staged
            # PSUM->SBUF copy before the DVE max was tried and is NOT
            # faster: the fp32 source blocks the scalar engine's 2x mode,
            # so the copy costs as much as the DVE op it would speed up.)
            if True:
                for k in range(NSHIFT):
                    shp = shpp.tile(
                        [128, CPRIME], F32, name=f"shp{k}", tag="shp"
                    )
                    for h in range(2):
                        sl = slice(h * 512, (h + 1) * 512)
                        nc.tensor.matmul(
                            shp[:, sl],
                            sh_ap(k),
                            L_ap(k, h, KS[k]),
                            start=True,
                            stop=True,
                        )
                        nc.vector.tensor_max(
                            L[k + 1][:, sl],
                            L_ap(k, h, KS[k + 1]),
                            shp[0 : KS[k + 1], sl],
                        )
                    emit_gathers(k, gi=0)
                emit_gathers(NSHIFT, gi=0)
            for k in range(NLEV):
                emit_gathers(k, gi=1)


            # TensorTensor may read only one PSUM operand: stage acc0 through
            # SBUF via the (otherwise idle) scalar engine, casting to bf16.
            # High priority so the ACT copy runs as soon as PSUM bank 0's
            # accumulation closes, overlapping the deferred gi=1 gathers.
            s1t = gwp.tile([T, CPRIME], BF16, name="s1t")
            ot = gwp.tile([T, CPRIME], BF16, name="ot")
            with tc.high_priority():
                for half in range(2):
                    sl = slice(half * 512, (half + 1) * 512)
                    nc.scalar.copy(out=s1t[:, sl], in_=p_acc[0][0:T, sl])
                    nc.vector.tensor_max(
                        ot[:, sl], s1t[:, sl], p_acc[1][0:T, sl]
                    )
                    eng = nc.sync if half == 0 else nc.scalar
                    eng.dma_start(out=out[:, sl], in_=ot[:, sl])

    nc.compile()
    return nc


def _host_windows(segments):
    """Replicates the reference's index math on segments[0]. Returns per half
    (lo, hi) clamped windows plus the empty mask."""
    seg = np.clip(segments.astype(np.float32), 0.0, 125.0)
    row = seg[0]  # [T, 4]
    s0 = np.floor(row[:, 0]).astype(np.int32)
    s1 = np.ceil(row[:, 1]).astype(np.int32)
    s1 = np.where(s0 == s1, s1 + 1, s1)
    e0 = np.floor(row[:, 2]).astype(np.int32)
    e1 = np.ceil(row[:, 3]).astype(np.int32)
    e0 = np.where(e0 == e1, e0 - 1, e0)

    halves = []
    for lo, hi in ((s0, s1), (e0, e1)):
        lo_c = np.maximum(lo, 0)
        hi_c = np.minimum(hi, T)
        empty = lo_c >= hi_c
        halves.append((lo_c, hi_c, empty))
    return halves


def _host_pk(segments):
    """Packed uint8 input tensors (fp8 one-hot bytes; pk0 also carries ft
    bytes which the caller fills per core)."""
    halves = _host_windows(segments)
    offs, nbytes = _w8_layout()
    one = mybir.dt.np(F8)(1.0).view(np.uint8)  # fp8e4 bit pattern of 1.0
    pk = [np.zeros((T, n), np.uint8) for n in nbytes]
    for k in range(NSHIFT):
        ti, o = offs[("sh", k)]
        s = 1 << k
        for j in range(KS[k + 1]):
            pk[ti][j + s, o + j] = one
    for h, (lo, hi, empty) in enumerate(halves):
        for t in range(T):
            if empty[t]:
                continue
            ln = int(hi[t] - lo[t])
            k = ln.bit_length() - 1
            a = int(lo[t])
            b = int(hi[t]) - (1 << k)
            ta, oa = offs[("g", 0, h, k)]
            tb, ob = offs[("g", 1, h, k)]
            pk[ta][a, oa + t] = one
            pk[tb][b, ob + t] = one
    return pk, halves


def _shard_feature(feature):
    """Core i gets batches [2i, 2i+2) as bf16 [T, CPRIME] with
    c' = half*512 + local_batch*256 + channel_within_half."""
    bf = mybir.dt.np(BF16)
    fts = []
    for i in range(NCORES):
        pair = feature[BPC * i : BPC * (i + 1)]
        arr = pair.reshape(BPC, 2, C, T)  # [b, h, c, j]
        arr = np.ascontiguousarray(arr.transpose(3, 1, 0, 2).reshape(T, CPRIME))
        fts.append(arr.astype(bf))
    return fts


def _unshard(results, halves):
    out = np.empty((B, C2, T), np.float32)
    for i in range(NCORES):
        r = np.asarray(results[i]["out"]).astype(np.float32)  # [T, CPRIME]
        arr = r.reshape(T, 2, BPC, C).transpose(2, 1, 3, 0)  # [b, h, c, t]
        out[BPC * i : BPC * (i + 1)] = arr.reshape(BPC, C2, T)
    neg = np.finfo(np.float32).min
    for h, (_, _, empty) in enumerate(halves):
        if empty.any():
            out[:, h * C : (h + 1) * C, empty] = neg
    return out


def kernel(feature, segments):
    global LAST_RESULTS
    feature = np.ascontiguousarray(feature, dtype=np.float32)
    segments = np.ascontiguousarray(segments, dtype=np.float32)

    if "nc" not in _CACHE:
        _CACHE["nc"] = _build_module()
    nc = _CACHE["nc"]

    pk, halves = _host_pk(segments)
    fts = _shard_feature(feature)

    in_maps = []
    for i in range(NCORES):
        pk0 = pk[0].copy()
        pk0[:, 0:FT_BYTES] = fts[i].view(np.uint8)
        in_maps.append({"pk0": pk0, "pk1": pk[1], "pk2": pk[2]})

    res = run_bass_kernel_spmd(nc, in_maps, list(range(NCORES)), trace=TRACE)
    LAST_RESULTS = res
    return _unshard(res.results, halves)
